# revision 19
# baseline (speedup 1.0000x reference)
"""LSTMCell (B=16384, IN=HID=512) on 8 TRN2 NeuronCores — v3.

Data-parallel over batch (2048 rows/core), weights replicated.
bf16 GEMM (the accuracy-safe PE floor: fp8 DoubleRow streams at 1
cycle/row on TRN2 hw, so a 3-pass fp8 split is 1.5x bf16 work).

vs the v1 baseline:
  - k-chunked DMA (128KB granularity) so the first matmul starts ~2-3us
    after launch instead of ~12us.
  - matmul loop is (r, g, k, nb): 4 batch-chunks stream per weight
    block, and redundant LDWEIGHTS are deleted post-schedule (verified
    on hw: the PE weight buffer persists across matmuls).
  - c_in / gates / cell-update / outputs in bf16: halves DMA traffic
    and doubles DVE throughput; everything stays hidden under the PE.
"""

import sys

sys.path.insert(0, "/opt/trn_rl_repo")

from contextlib import ExitStack

import ml_dtypes
import numpy as np

import concourse.bass as bass  # noqa: F401
import concourse.mybir as mybir
import concourse.tile as tile
from concourse import bacc
from concourse.bass_utils import run_bass_kernel_spmd

B_FULL, IN, HID = 16384, 512, 512
NCORES = 8
BL = B_FULL // NCORES  # 2048 batch rows per core
JW = 512               # batch columns per chunk (matmul free dim)
P = 128

BF16 = mybir.dt.bfloat16
F32 = mybir.dt.float32
AF = mybir.ActivationFunctionType
BF16_NP = ml_dtypes.bfloat16

NK = (IN + HID) // P   # 8  k-chunks of the contraction dim
NR = HID // P          # 4  row-blocks of H per gate
NM = 4 * HID // P      # 16 gate-row blocks total (i,g,f,o order)

WARMUP_MM = 14


def dedup_ldweights(nc):
    """Delete InstLdweights whose weights AP matches the immediately
    preceding LDWEIGHTS on the PE queue. Non-self-loading matmuls keep
    using the loaded weights (verified on hw). Deps of the removed LW
    are merged into the next PE instruction."""
    removed = 0
    for f in nc.m.functions:
        for b in f.blocks:
            insts = b.instructions
            last_key = None
            to_remove = []
            for idx, inst in enumerate(insts):
                if type(inst).__name__ == "InstLdweights":
                    key = str(inst.ins[0])
                    if key == last_key:
                        to_remove.append(idx)
                    last_key = key
            for idx in reversed(to_remove):
                lw = insts[idx]
                nxt = None
                for j in range(idx + 1, len(insts)):
                    if insts[j].engine == lw.engine:
                        nxt = insts[j]
                        break
                if nxt is not None:
                    nxt.merge_dependencies_from(lw)
                insts.remove(lw)
                removed += 1
    return removed


def build_nc(bl=BL):
    """Build the single-core Bass program (SPMD-replicated across cores)."""
    nbn = bl // JW
    nc = bacc.Bacc("TRN2", target_bir_lowering=False, debug=False)

    xh_in = nc.dram_tensor("xh_in", [NK, P, nbn, JW], BF16, kind="ExternalInput")
    wt_in = nc.dram_tensor("wt_in", [NK, P, 4 * HID], BF16, kind="ExternalInput")
    bias_in = nc.dram_tensor("bias_in", [P, NM], F32, kind="ExternalInput")
    c_in = nc.dram_tensor("c_in", [nbn, P, NR, JW], BF16, kind="ExternalInput")
    h_out = nc.dram_tensor("h_out", [nbn, NR, P, JW], BF16, kind="ExternalOutput")
    c_out = nc.dram_tensor("c_out", [nbn, NR, P, JW], BF16, kind="ExternalOutput")

    with ExitStack() as ctx:
        tc = ctx.enter_context(tile.TileContext(nc))
        wpool = ctx.enter_context(tc.tile_pool(name="w", bufs=1))
        xpool = ctx.enter_context(tc.tile_pool(name="xh", bufs=1))
        cpool = ctx.enter_context(tc.tile_pool(name="cin", bufs=1))
        gpool = ctx.enter_context(tc.tile_pool(name="gates", bufs=2))
        opool = ctx.enter_context(tc.tile_pool(name="outs", bufs=3))
        pspool = ctx.enter_context(tc.tile_pool(name="ps", bufs=1, space="PSUM"))

        # PE HAM warmup: bridge the PE from program start to first-data
        # (~16.5us) so real matmuls start without an idle-induced
        # re-ramp; ~427ns each at the gated mid clock.
        wu = wpool.tile([P, JW], BF16, tag="wu", name="wu")
        nc.vector.memset(wu[:], 0.0)
        wu_ps = pspool.tile([P, JW], F32, tag="ps0_0", name="wu_ps")
        for i in range(WARMUP_MM):
            nc.tensor.matmul(wu_ps[:], wu[:, (i % 2) * P : (i % 2 + 1) * P],
                             wu[:], start=True, stop=True)

        # Input DMAs, k-major so the first (g, k) matmul group can start
        # as soon as the first 128KB chunks land.
        #   gpsimd queue: weights (k-slices) + bias, then c_in
        #   sync queue:   xh chunks (k-major)
        #   scalar queue: outputs
        # gpsimd queue: bias, weights (k-order), then c_in
        # sync queue:   xh chunks (k-order)
        bias_t = wpool.tile([P, NM], F32, tag="bias", name="bias")
        nc.gpsimd.dma_start(bias_t[:], bias_in[:])
        wts = []
        for k in range(NK):
            wt = wpool.tile([P, 4 * HID], BF16, tag=f"w{k}", name=f"w{k}")
            nc.gpsimd.dma_start(wt[:], wt_in[k])
            wts.append(wt)
        xh_big = xpool.tile([P, NK, nbn, JW], BF16, tag="xh", name="xh")
        for k in range(NK):
            nc.sync.dma_start(xh_big[:, k], xh_in[k])

        c_big = cpool.tile([P, nbn, NR, JW], BF16, tag="c", name="c")
        for nb in range(nbn):
            nc.gpsimd.dma_start(c_big[:, nb], c_in[nb])

        for r in range(NR):
            gates = [[None] * nbn for _ in range(4)]
            tchs = [None] * nbn
            for g in range(4):
                m = g * NR + r
                ms = slice(m * P, (m + 1) * P)
                ps = [
                    pspool.tile([P, JW], F32, tag=f"ps{g % 2}_{nb}",
                                name=f"ps{g % 2}_{nb}")
                    for nb in range(nbn)
                ]
                for k in range(NK):
                    if r == 0 and g == 0 and k > 0:
                        # accumulate zeros into the open chains while the
                        # next input chunks stream in: keeps the PE clock
                        # from gating down during arrival-paced stalls
                        for i in range(5):
                            nc.tensor.matmul(
                                ps[i % nbn][:],
                                wu[:, (i % 2) * P : (i % 2 + 1) * P],
                                wu[:], start=False, stop=False,
                                skip_group_check=True)
                    for nb in range(nbn):
                        nc.tensor.matmul(
                            ps[nb][:],
                            wts[k][:, ms],
                            xh_big[:, k, nb, :],
                            start=(k == 0),
                            stop=(k == NK - 1),
                        )
                func = AF.Tanh if g == 1 else AF.Sigmoid
                for nb in range(nbn):
                    gt = gpool.tile([P, JW], F32, tag=f"g{g}_{nb}")
                    nc.scalar.activation(
                        gt[:], ps[nb][:], func, bias=bias_t[:, m : m + 1]
                    )
                    gates[g][nb] = gt
                if g == 2:
                    # i, g, f are ready: finish the cell state and its tanh
                    # while the o-gate matmuls run; only hn remains after o.
                    for nb in range(nbn):
                        it, gt, ft = (gates[gg][nb] for gg in range(3))
                        t1 = gpool.tile([P, JW], F32, tag="t1")
                        t2 = gpool.tile([P, JW], F32, tag="t2")
                        cn = opool.tile([P, JW], BF16, tag="cn")
                        tch = gpool.tile([P, JW], BF16, tag=f"tch{nb}")
                        nc.vector.tensor_mul(t1[:], it[:], gt[:])
                        nc.vector.tensor_mul(t2[:], ft[:], c_big[:, nb, r, :])
                        nc.vector.tensor_add(cn[:], t1[:], t2[:])
                        nc.scalar.dma_start(c_out[nb, r], cn[:])
                        nc.scalar.activation(tch[:], cn[:], AF.Tanh)
                        tchs[nb] = tch
            for nb in range(nbn):
                hn = opool.tile([P, JW], BF16, tag="hn")
                nc.vector.tensor_mul(hn[:], gates[3][nb][:], tchs[nb][:])
                nc.scalar.dma_start(h_out[nb, r], hn[:])
    dedup_ldweights(nc)
    nc.compile()
    return nc


def prep_shared(Wxi, Wxg, Wxf, Wxo, Whi, Whg, Whf, Who, bias_sum):
    """wt_in [NK,P,4H] bf16 and bias_in [P,NM] f32 (gate order i,g,f,o)."""
    Wx = np.concatenate([Wxi, Wxg, Wxf, Wxo], axis=0)  # [4H, IN]
    Wh = np.concatenate([Whi, Whg, Whf, Who], axis=0)  # [4H, HID]
    WT = np.concatenate([Wx.T, Wh.T], axis=0)          # [K=1024, 4H]
    wt_arr = np.ascontiguousarray(
        WT.reshape(NK, P, 4 * HID).astype(BF16_NP)
    )
    bias_arr = np.ascontiguousarray(
        bias_sum.reshape(NM, P).T.astype(np.float32)
    )
    return wt_arr, bias_arr


def prep_core(x_s, h_s, c_s):
    """Per-core xh_in [NK,nb,P,JW] bf16 and c_in [nb,NR,P,JW] bf16."""
    bl = x_s.shape[0]
    nbn = bl // JW
    xhT = np.concatenate([x_s, h_s], axis=1).T  # [K=1024, bl]
    xh_arr = np.ascontiguousarray(
        xhT.reshape(NK, P, nbn, JW).astype(BF16_NP)
    )
    cT = c_s.T  # [HID, bl]
    c_arr = np.ascontiguousarray(
        cT.reshape(NR, P, nbn, JW).transpose(2, 1, 0, 3).astype(BF16_NP)
    )
    return xh_arr, c_arr


def post_core(arr):
    """[nb,NR,P,JW] -> [bl, HID] f32"""
    arr = np.asarray(arr).astype(np.float32)
    nbn = arr.size // (NR * P * JW)
    arr = arr.reshape(nbn, NR, P, JW)
    return arr.transpose(0, 3, 1, 2).reshape(nbn * JW, HID)


_NC_CACHE = {}


def _get_nc(bl=BL):
    if bl not in _NC_CACHE:
        _NC_CACHE[bl] = build_nc(bl)
    return _NC_CACHE[bl]


def make_in_maps(x, h, c, Wxi, bxi, Wxo, bxo, Wxf, bxf, Wxg, bxg,
                 Whi, bhi, Who, bho, Whf, bhf, Whg, bhg, ncores=NCORES):
    bias_sum = np.concatenate(
        [bxi + bhi, bxg + bhg, bxf + bhf, bxo + bho], axis=0
    ).astype(np.float32)
    wt_arr, bias_arr = prep_shared(Wxi, Wxg, Wxf, Wxo, Whi, Whg, Whf, Who, bias_sum)
    bl = x.shape[0] // ncores
    in_maps = []
    for i in range(ncores):
        s = slice(i * bl, (i + 1) * bl)
        xh_arr, c_arr = prep_core(
            np.asarray(x[s], np.float32),
            np.asarray(h[s], np.float32),
            np.asarray(c[s], np.float32),
        )
        in_maps.append(
            {"xh_in": xh_arr, "wt_in": wt_arr, "bias_in": bias_arr, "c_in": c_arr}
        )
    return in_maps


def kernel(x, h, c, Wxi, bxi, Wxo, bxo, Wxf, bxf, Wxg, bxg,
           Whi, bhi, Who, bho, Whf, bhf, Whg, bhg):
    args = dict(
        x=np.asarray(x, np.float32), h=np.asarray(h, np.float32),
        c=np.asarray(c, np.float32),
        Wxi=np.asarray(Wxi, np.float32), bxi=np.asarray(bxi, np.float32),
        Wxo=np.asarray(Wxo, np.float32), bxo=np.asarray(bxo, np.float32),
        Wxf=np.asarray(Wxf, np.float32), bxf=np.asarray(bxf, np.float32),
        Wxg=np.asarray(Wxg, np.float32), bxg=np.asarray(bxg, np.float32),
        Whi=np.asarray(Whi, np.float32), bhi=np.asarray(bhi, np.float32),
        Who=np.asarray(Who, np.float32), bho=np.asarray(bho, np.float32),
        Whf=np.asarray(Whf, np.float32), bhf=np.asarray(bhf, np.float32),
        Whg=np.asarray(Whg, np.float32), bhg=np.asarray(bhg, np.float32),
    )
    in_maps = make_in_maps(**args)
    nc = _get_nc(BL)
    res = run_bass_kernel_spmd(nc, in_maps, core_ids=list(range(NCORES)))
    h_new = np.empty((B_FULL, HID), np.float32)
    c_new = np.empty((B_FULL, HID), np.float32)
    for i in range(NCORES):
        s = slice(i * BL, (i + 1) * BL)
        h_new[s] = post_core(res.results[i]["h_out"])
        c_new[s] = post_core(res.results[i]["c_out"])
    return (h_new, c_new)


# revision 20
# speedup vs baseline: 1.0341x; 1.0341x over previous
"""LSTMCell (B=16384, IN=HID=512) on 8 TRN2 NeuronCores — v3.

Data-parallel over batch (2048 rows/core), weights replicated.
bf16 GEMM (the accuracy-safe PE floor: fp8 DoubleRow streams at 1
cycle/row on TRN2 hw, so a 3-pass fp8 split is 1.5x bf16 work).

vs the v1 baseline (135.7us):
  - matmul loop is (r, g, k, nb): 4 batch-chunks stream per loaded
    weight block, and the redundant LDWEIGHTS are deleted pre-compile
    (the PE weight buffer persists across matmuls; verified on hw).
    With the rhs sliced from ONE resident SBUF tile the matmul cadence
    is 216ns (512-cycle stream) instead of the baseline's 260ns.
  - xh lives in a single [P, NK, nbn, JW] SBUF tile filled by 8 large
    k-major DMAs (per-dma_start issue cost is ~0.7us regardless of
    size); weights k-slices + bias on the gpsimd queue in parallel.
  - c_in and h/c outputs in bf16 (halves that DMA traffic); gates and
    products stay fp32; cell state + tanh are computed as soon as the
    f-gate lands so only hn = o*tanh(c) trails the last matmul.
  - fp8e4 DoubleRow was measured to stream at 1 cycle/row on hw (2x
    MACs/cycle, not the cost model's 0.5 cyc/row), so an accurate
    3-pass fp8 hi/lo split is 1.5x the bf16 work: bf16 1-pass is the
    accuracy-safe PE floor (109.2us/core of streams).
"""

import sys

sys.path.insert(0, "/opt/trn_rl_repo")

from contextlib import ExitStack

import ml_dtypes
import numpy as np

import concourse.bass as bass  # noqa: F401
import concourse.mybir as mybir
import concourse.tile as tile
from concourse import bacc
from concourse.bass_utils import run_bass_kernel_spmd

B_FULL, IN, HID = 16384, 512, 512
NCORES = 8
BL = B_FULL // NCORES  # 2048 batch rows per core
JW = 512               # batch columns per chunk (matmul free dim)
P = 128

BF16 = mybir.dt.bfloat16
F32 = mybir.dt.float32
AF = mybir.ActivationFunctionType
BF16_NP = ml_dtypes.bfloat16

NK = (IN + HID) // P   # 8  k-chunks of the contraction dim
NR = HID // P          # 4  row-blocks of H per gate
NM = 4 * HID // P      # 16 gate-row blocks total (i,g,f,o order)

WARMUP_MM = 14


def dedup_ldweights(nc):
    """Delete InstLdweights whose weights AP matches the immediately
    preceding LDWEIGHTS on the PE queue. Non-self-loading matmuls keep
    using the loaded weights (verified on hw). Deps of the removed LW
    are merged into the next PE instruction."""
    removed = 0
    for f in nc.m.functions:
        for b in f.blocks:
            insts = b.instructions
            last_key = None
            to_remove = []
            for idx, inst in enumerate(insts):
                if type(inst).__name__ == "InstLdweights":
                    key = str(inst.ins[0])
                    if key == last_key:
                        to_remove.append(idx)
                    last_key = key
            for idx in reversed(to_remove):
                lw = insts[idx]
                nxt = None
                for j in range(idx + 1, len(insts)):
                    if insts[j].engine == lw.engine:
                        nxt = insts[j]
                        break
                if nxt is not None:
                    nxt.merge_dependencies_from(lw)
                insts.remove(lw)
                removed += 1
    return removed


def build_nc(bl=BL):
    """Build the single-core Bass program (SPMD-replicated across cores)."""
    nbn = bl // JW
    nc = bacc.Bacc("TRN2", target_bir_lowering=False, debug=False)

    xh_in = nc.dram_tensor("xh_in", [NK, P, nbn, JW], BF16, kind="ExternalInput")
    wt_in = nc.dram_tensor("wt_in", [NK, P, 4 * HID], BF16, kind="ExternalInput")
    bias_in = nc.dram_tensor("bias_in", [P, NM], F32, kind="ExternalInput")
    c_in = nc.dram_tensor("c_in", [nbn, P, NR, JW], BF16, kind="ExternalInput")
    h_out = nc.dram_tensor("h_out", [nbn, NR, P, JW], BF16, kind="ExternalOutput")
    c_out = nc.dram_tensor("c_out", [nbn, NR, P, JW], BF16, kind="ExternalOutput")

    with ExitStack() as ctx:
        tc = ctx.enter_context(tile.TileContext(nc))
        wpool = ctx.enter_context(tc.tile_pool(name="w", bufs=1))
        xpool = ctx.enter_context(tc.tile_pool(name="xh", bufs=1))
        cpool = ctx.enter_context(tc.tile_pool(name="cin", bufs=1))
        gpool = ctx.enter_context(tc.tile_pool(name="gates", bufs=2))
        opool = ctx.enter_context(tc.tile_pool(name="outs", bufs=3))
        pspool = ctx.enter_context(tc.tile_pool(name="ps", bufs=1, space="PSUM"))

        # PE HAM warmup: bridge the PE from program start to first-data
        # (~16.5us) so real matmuls start without an idle-induced
        # re-ramp; ~427ns each at the gated mid clock.
        wu = wpool.tile([P, JW], BF16, tag="wu", name="wu")
        nc.vector.memset(wu[:], 0.0)
        wu_ps = pspool.tile([P, JW], F32, tag="ps0_0", name="wu_ps")
        for i in range(WARMUP_MM):
            nc.tensor.matmul(wu_ps[:], wu[:, (i % 2) * P : (i % 2 + 1) * P],
                             wu[:], start=True, stop=True)

        # Input DMAs, k-major so the first (g, k) matmul group can start
        # as soon as the first 128KB chunks land.
        #   gpsimd queue: weights (k-slices) + bias, then c_in
        #   sync queue:   xh chunks (k-major)
        #   scalar queue: outputs
        # gpsimd queue: bias, weights (k-order), then c_in
        # sync queue:   xh chunks (k-order)
        bias_t = wpool.tile([P, NM], F32, tag="bias", name="bias")
        nc.gpsimd.dma_start(bias_t[:], bias_in[:])
        wts = []
        for k in range(NK):
            wt = wpool.tile([P, 4 * HID], BF16, tag=f"w{k}", name=f"w{k}")
            nc.gpsimd.dma_start(wt[:], wt_in[k])
            wts.append(wt)
        xh_big = xpool.tile([P, NK, nbn, JW], BF16, tag="xh", name="xh")
        for k in range(NK):
            nc.sync.dma_start(xh_big[:, k], xh_in[k])

        c_big = cpool.tile([P, nbn, NR, JW], BF16, tag="c", name="c")
        for nb in range(nbn):
            nc.gpsimd.dma_start(c_big[:, nb], c_in[nb])

        for r in range(NR):
            gates = [[None] * nbn for _ in range(4)]
            tchs = [None] * nbn
            for g in range(4):
                m = g * NR + r
                ms = slice(m * P, (m + 1) * P)
                ps = [
                    pspool.tile([P, JW], F32, tag=f"ps{g % 2}_{nb}",
                                name=f"ps{g % 2}_{nb}")
                    for nb in range(nbn)
                ]
                for k in range(NK):
                    for nb in range(nbn):
                        nc.tensor.matmul(
                            ps[nb][:],
                            wts[k][:, ms],
                            xh_big[:, k, nb, :],
                            start=(k == 0),
                            stop=(k == NK - 1),
                        )
                func = AF.Tanh if g == 1 else AF.Sigmoid
                for nb in range(nbn):
                    gt = gpool.tile([P, JW], F32, tag=f"g{g}_{nb}")
                    nc.scalar.activation(
                        gt[:], ps[nb][:], func, bias=bias_t[:, m : m + 1]
                    )
                    gates[g][nb] = gt
                if g == 2:
                    # i, g, f are ready: finish the cell state and its tanh
                    # while the o-gate matmuls run; only hn remains after o.
                    for nb in range(nbn):
                        it, gt, ft = (gates[gg][nb] for gg in range(3))
                        t1 = gpool.tile([P, JW], F32, tag="t1")
                        t2 = gpool.tile([P, JW], F32, tag="t2")
                        cn = opool.tile([P, JW], BF16, tag="cn")
                        tch = gpool.tile([P, JW], BF16, tag=f"tch{nb}")
                        nc.vector.tensor_mul(t1[:], it[:], gt[:])
                        nc.vector.tensor_mul(t2[:], ft[:], c_big[:, nb, r, :])
                        nc.vector.tensor_add(cn[:], t1[:], t2[:])
                        nc.scalar.dma_start(c_out[nb, r], cn[:])
                        nc.scalar.activation(tch[:], cn[:], AF.Tanh)
                        tchs[nb] = tch
            for nb in range(nbn):
                hn = opool.tile([P, JW], BF16, tag="hn")
                nc.vector.tensor_mul(hn[:], gates[3][nb][:], tchs[nb][:])
                nc.scalar.dma_start(h_out[nb, r], hn[:])
    dedup_ldweights(nc)
    nc.compile()
    return nc


def prep_shared(Wxi, Wxg, Wxf, Wxo, Whi, Whg, Whf, Who, bias_sum):
    """wt_in [NK,P,4H] bf16 and bias_in [P,NM] f32 (gate order i,g,f,o)."""
    Wx = np.concatenate([Wxi, Wxg, Wxf, Wxo], axis=0)  # [4H, IN]
    Wh = np.concatenate([Whi, Whg, Whf, Who], axis=0)  # [4H, HID]
    WT = np.concatenate([Wx.T, Wh.T], axis=0)          # [K=1024, 4H]
    wt_arr = np.ascontiguousarray(
        WT.reshape(NK, P, 4 * HID).astype(BF16_NP)
    )
    bias_arr = np.ascontiguousarray(
        bias_sum.reshape(NM, P).T.astype(np.float32)
    )
    return wt_arr, bias_arr


def prep_core(x_s, h_s, c_s):
    """Per-core xh_in [NK,nb,P,JW] bf16 and c_in [nb,NR,P,JW] bf16."""
    bl = x_s.shape[0]
    nbn = bl // JW
    xhT = np.concatenate([x_s, h_s], axis=1).T  # [K=1024, bl]
    xh_arr = np.ascontiguousarray(
        xhT.reshape(NK, P, nbn, JW).astype(BF16_NP)
    )
    cT = c_s.T  # [HID, bl]
    c_arr = np.ascontiguousarray(
        cT.reshape(NR, P, nbn, JW).transpose(2, 1, 0, 3).astype(BF16_NP)
    )
    return xh_arr, c_arr


def post_core(arr):
    """[nb,NR,P,JW] -> [bl, HID] f32"""
    arr = np.asarray(arr).astype(np.float32)
    nbn = arr.size // (NR * P * JW)
    arr = arr.reshape(nbn, NR, P, JW)
    return arr.transpose(0, 3, 1, 2).reshape(nbn * JW, HID)


_NC_CACHE = {}


def _get_nc(bl=BL):
    if bl not in _NC_CACHE:
        _NC_CACHE[bl] = build_nc(bl)
    return _NC_CACHE[bl]


def make_in_maps(x, h, c, Wxi, bxi, Wxo, bxo, Wxf, bxf, Wxg, bxg,
                 Whi, bhi, Who, bho, Whf, bhf, Whg, bhg, ncores=NCORES):
    bias_sum = np.concatenate(
        [bxi + bhi, bxg + bhg, bxf + bhf, bxo + bho], axis=0
    ).astype(np.float32)
    wt_arr, bias_arr = prep_shared(Wxi, Wxg, Wxf, Wxo, Whi, Whg, Whf, Who, bias_sum)
    bl = x.shape[0] // ncores
    in_maps = []
    for i in range(ncores):
        s = slice(i * bl, (i + 1) * bl)
        xh_arr, c_arr = prep_core(
            np.asarray(x[s], np.float32),
            np.asarray(h[s], np.float32),
            np.asarray(c[s], np.float32),
        )
        in_maps.append(
            {"xh_in": xh_arr, "wt_in": wt_arr, "bias_in": bias_arr, "c_in": c_arr}
        )
    return in_maps


def kernel(x, h, c, Wxi, bxi, Wxo, bxo, Wxf, bxf, Wxg, bxg,
           Whi, bhi, Who, bho, Whf, bhf, Whg, bhg):
    args = dict(
        x=np.asarray(x, np.float32), h=np.asarray(h, np.float32),
        c=np.asarray(c, np.float32),
        Wxi=np.asarray(Wxi, np.float32), bxi=np.asarray(bxi, np.float32),
        Wxo=np.asarray(Wxo, np.float32), bxo=np.asarray(bxo, np.float32),
        Wxf=np.asarray(Wxf, np.float32), bxf=np.asarray(bxf, np.float32),
        Wxg=np.asarray(Wxg, np.float32), bxg=np.asarray(bxg, np.float32),
        Whi=np.asarray(Whi, np.float32), bhi=np.asarray(bhi, np.float32),
        Who=np.asarray(Who, np.float32), bho=np.asarray(bho, np.float32),
        Whf=np.asarray(Whf, np.float32), bhf=np.asarray(bhf, np.float32),
        Whg=np.asarray(Whg, np.float32), bhg=np.asarray(bhg, np.float32),
    )
    in_maps = make_in_maps(**args)
    nc = _get_nc(BL)
    res = run_bass_kernel_spmd(nc, in_maps, core_ids=list(range(NCORES)))
    h_new = np.empty((B_FULL, HID), np.float32)
    c_new = np.empty((B_FULL, HID), np.float32)
    for i in range(NCORES):
        s = slice(i * BL, (i + 1) * BL)
        h_new[s] = post_core(res.results[i]["h_out"])
        c_new[s] = post_core(res.results[i]["c_out"])
    return (h_new, c_new)
